# revision 25
# baseline (speedup 1.0000x reference)
"""AdaptiveCLPL loss on 8 TRN2 NeuronCores (Bass/Tile) — v6.

loss = mean_b [ psi(avg_cand) + sum_head psi(-l)*(1-mask) + ts*sum_samp psi(-l)*(1-is_cand) ]
with psi(u) = softplus(-u) = Ln(Exp(-u)+1) (no native softplus table).

Decomposition (only term1 is per-row nonlinear; everything else sums):
  total = sum_b softplus(-avg_b)
        + [sum_{head block} softplus(l)    - sum_k uniq*inhead*softplus(l_cand)]
        + ts*[sum_{sampled rows} softplus(l) - sum_k uniq*mult*softplus(l_cand)]

Per-core layout: transposed batch shard lT = logits[rows_perm].T in BF16
([C, RB] row-major); every lT row is a 512B chunk addressed by class (bf16
halves both the gather wire and the DVE extraction; the 2e-2 tolerance has
orders of magnitude of headroom). Candidate values come from dma_gather (one
descriptor per candidate). Key points:
  - overlapping int16 windows [0,32768) and [C-32768, C): candidates in the
    overlap go to either window, so every partition holds EXACTLY nj0+nj1
    candidate slots -> zero descriptor padding (2560 descriptors, the floor).
  - sampled rows ride the window gather calls as extra trailing indices,
    replacing the slow gpsimd indirect DMA.
  - 3 gather calls (w0 | w1a | w1b) across 2 SWDGE queues: each call's
    end-doorbell releases its wire while the next call's descriptors
    generate, so extraction pipelines with generation.
  - a 16-idx dummy gather issued first pays the gpsimd 'mlp' library IRAM
    load while the idx/aux DMAs are in flight; the 2MB head DMA is gated
    behind it so the library image isn't bandwidth-starved. The head input
    is reshaped host-side to 128 partitions so its DMA spreads across all
    16 SDMA engines (a [125, *] shape lands on only 5).
  - act tables are doctored at compile time so Exp and Ln resolve to the one
    table set that contains both -> one ACT_TABLE_LOAD, primed early by a
    dummy activation.
  - rows are packed 2 per partition; the shard column of row (p,g) is 2p+g.
"""

import numpy as np
import ml_dtypes

BF16 = ml_dtypes.bfloat16

B, C, K = 2048, 50000, 10
HEAD, S = 2000, 100
TSCALE = float(C - HEAD) / float(S)  # 480.0
NCORES = 8
RB = B // NCORES  # 256 rows per core
P = 128
ES = 256          # chunk = one lT row (512B in bf16)
WIN = 32768
LO1 = C - WIN     # 17232; window1 = [LO1, C)
GMAX = 2          # exactly 2 rows per partition
HW_ = HEAD * RB // P  # 4000 head elements per partition

_CACHE = {}


def _pack_rows(h0, h1, nj_target, rng):
    """Pair 2*P rows into P partitions s.t. per-partition hard-window counts
    stay <= nj_target. Returns part[r] in [0,P)."""
    nrows = len(h0)
    order = np.argsort(-h0, kind="stable")
    part = np.zeros(nrows, np.int64)
    for i in range(P):
        part[order[i]] = i
        part[order[nrows - 1 - i]] = i
    H0 = np.bincount(part, weights=h0, minlength=P)
    H1 = np.bincount(part, weights=h1, minlength=P)

    def viol(a0, a1):
        return max(a0 - nj_target, 0) + max(a1 - nj_target, 0)

    cur = sum(viol(H0[p], H1[p]) for p in range(P))
    it = 0
    while cur > 0 and it < 20000:
        it += 1
        a, b = rng.integers(0, nrows, 2)
        pa, pb = part[a], part[b]
        if pa == pb:
            continue
        old = viol(H0[pa], H1[pa]) + viol(H0[pb], H1[pb])
        H0[pa] += h0[b] - h0[a]; H1[pa] += h1[b] - h1[a]
        H0[pb] += h0[a] - h0[b]; H1[pb] += h1[a] - h1[b]
        new = viol(H0[pa], H1[pa]) + viol(H0[pb], H1[pb])
        if new <= old:
            part[a], part[b] = pb, pa
            cur += new - old
        else:
            H0[pa] -= h0[b] - h0[a]; H1[pa] -= h1[b] - h1[a]
            H0[pb] -= h0[a] - h0[b]; H1[pb] -= h1[a] - h1[b]
    return part, cur == 0


def prep_inputs(logits, candidates, sampled_indices):
    """Full inputs -> (in_maps, meta). Host work is sharding + index math only."""
    logits = np.asarray(logits)
    candidates = np.asarray(candidates)
    sampled_indices = np.asarray(sampled_indices)
    assert logits.shape == (B, C) and candidates.shape == (B, K)
    srow = (HEAD + sampled_indices.astype(np.int64)).astype(np.int64)  # [S]
    svals, scounts = np.unique(srow, return_counts=True)
    smult = dict(zip(svals.tolist(), scounts.tolist()))

    # sampled rows -> windows (balance the flexible ones)
    s_w = np.where(srow < LO1, 0, np.where(srow >= WIN, 1, -1))
    flex = np.where(s_w < 0)[0]
    n0 = int((s_w == 0).sum())
    n1 = int((s_w == 1).sum())
    for j in flex:
        if n0 <= n1:
            s_w[j] = 0; n0 += 1
        else:
            s_w[j] = 1; n1 += 1
    ns0, ns1 = n0, n1
    sidx_w = [srow[s_w == 0] - 0, srow[s_w == 1] - LO1]

    rng = np.random.default_rng(12345)
    cores = []
    nj_need = [1, 1]
    for i in range(NCORES):
        rows = slice(i * RB, (i + 1) * RB)
        cand = candidates[rows].astype(np.int64)          # [RB, K]
        valid = cand >= 0
        uniq = valid.copy()
        for k in range(1, K):
            dup = (cand[:, :k] == cand[:, k:k + 1]).any(axis=1)
            uniq[:, k] &= ~dup
        uniqf = uniq.astype(np.float32)
        cnt = np.maximum((uniq & valid).sum(axis=1), 1).astype(np.float32)
        inhead = (cand < HEAD).astype(np.float32)
        mult = np.vectorize(lambda c: smult.get(int(c), 0))(cand).astype(np.float32)
        wcorr_rk = -uniqf * (inhead + TSCALE * mult)      # [RB, K]

        h0 = (valid & (cand < LO1)).sum(axis=1)
        h1 = (valid & (cand >= WIN)).sum(axis=1)
        part, ok = _pack_rows(h0.astype(np.int64), h1.astype(np.int64), K, rng)
        grp = np.zeros(RB, np.int64)
        seen = {}
        for r in range(RB):
            p = int(part[r])
            grp[r] = seen.get(p, 0)
            seen[p] = grp[r] + 1
        assert max(seen.values()) <= GMAX

        # window assignment per candidate
        cw = np.full((RB, K), -1, np.int64)
        cw[valid & (cand < LO1)] = 0
        cw[valid & (cand >= WIN)] = 1
        H0 = np.bincount(part, weights=(cw == 0).sum(1), minlength=P).astype(np.int64)
        for r in range(RB):
            p = int(part[r])
            for k in range(K):
                if valid[r, k] and cw[r, k] < 0:
                    if H0[p] < K:
                        cw[r, k] = 0; H0[p] += 1
                    else:
                        cw[r, k] = 1
        W0c = np.bincount(part, weights=(cw == 0).sum(1), minlength=P).astype(np.int64)
        W1c = np.bincount(part, weights=(cw == 1).sum(1), minlength=P).astype(np.int64)
        nj_need[0] = max(nj_need[0], int(W0c.max()))
        nj_need[1] = max(nj_need[1], int(W1c.max()))
        cores.append((cand, valid, uniqf, cnt, wcorr_rk, part, grp, cw))

    nj0, nj1 = nj_need
    njtot = nj0 + nj1
    meta = (nj0, nj1, ns0, ns1)

    # gather calls: [w0a] [w1a] [w0b cand+samp0] [w1b cand+samp1]
    nj0a = (nj0 * 7 + 5) // 10
    nj0b = nj0 - nj0a
    nj1a = (nj1 * 7 + 5) // 10
    nj1b = nj1 - nj1a
    ni = [nj0a * P, nj1a * P, nj0b * P + ns0, nj1b * P + ns1]
    ncols = [(-(-n // 16)) for n in ni]
    AUXW = njtot + GMAX * njtot + GMAX          # f32: wcorr, wg, rcnt
    AUXHW = njtot + ES                          # bf16: offt, iota

    in_maps = []
    for i in range(NCORES):
        cand, valid, uniqf, cnt, wcorr_rk, part, grp, cw = cores[i]
        rows = slice(i * RB, (i + 1) * RB)
        col = (2 * part + grp).astype(np.int64)
        perm = np.zeros(RB, np.int64)
        perm[col] = np.arange(RB)
        lT = np.ascontiguousarray(
            logits[rows][perm].T.astype(np.float32).astype(BF16))
        lH = np.ascontiguousarray(lT[:HEAD].reshape(P, HW_))

        idxs = [np.zeros(nj0 * P, np.int16), np.zeros(nj1 * P, np.int16)]
        offt = np.full((P, njtot), -1.0, np.float32)
        wcorr = np.zeros((P, njtot), np.float32)
        wg = np.zeros((P, GMAX * njtot), np.float32)
        fill = np.zeros((P, 2), np.int64)
        base_j = [0, nj0]
        base_lo = [0, LO1]
        for r in range(RB):
            p, g = int(part[r]), int(grp[r])
            for k in range(K):
                if not valid[r, k]:
                    continue
                w = int(cw[r, k])
                j = int(fill[p, w]); fill[p, w] += 1
                idxs[w][j * P + p] = cand[r, k] - base_lo[w]
                jj = base_j[w] + j
                offt[p, jj] = float(col[r])
                wcorr[p, jj] = wcorr_rk[r, k]
                wg[p, g * njtot + jj] = uniqf[r, k]
        rcnt = np.zeros((P, GMAX), np.float32)
        rcnt[part, grp] = 1.0 / cnt
        iota = np.broadcast_to(np.arange(ES, dtype=np.float32), (P, ES)).copy()

        # per-call idx lists
        lists = [
            idxs[0][:nj0a * P],
            idxs[1][:nj1a * P],
            np.concatenate([idxs[0][nj0a * P:], sidx_w[0].astype(np.int16)]),
            np.concatenate([idxs[1][nj1a * P:], sidx_w[1].astype(np.int16)]),
        ]
        cols_out = []
        for li, n_c in zip(lists, ncols):
            flat = np.zeros(n_c * 16, np.int16)
            flat[:len(li)] = li
            wrapped = flat.reshape(n_c, 16).T
            cols_out.append(np.tile(wrapped, (8, 1)))
        idx16 = np.ascontiguousarray(np.concatenate(cols_out, axis=1))
        assert idx16.shape == (P, sum(ncols))

        auxcat = np.ascontiguousarray(np.concatenate(
            [wcorr, wg, rcnt], axis=1))
        assert auxcat.shape == (P, AUXW)
        auxh = np.ascontiguousarray(np.concatenate(
            [offt, iota], axis=1).astype(BF16))
        assert auxh.shape == (P, AUXHW)
        in_maps.append({"lT": lT, "lH": lH, "idx16": idx16,
                        "aux": auxcat, "auxh": auxh})
    return in_maps, meta


def _act_table_patch():
    """Context manager: make Exp and Ln resolve only to the one act-func set
    ('natural_log_exp_and_others') that holds both, so the compile-time table
    placement emits a single ACT_TABLE_LOAD instead of swapping per phase."""
    import contextlib as _ctl
    from concourse import hw_specs, mybir
    from concourse import bacc as _bacc

    @_ctl.contextmanager
    def ctx():
        real = hw_specs.get_activation_tables
        AF = mybir.ActivationFunctionType

        def doctored(arch):
            tabs = {k: set(v) for k, v in real(arch).items()}
            if any(AF.Exp in v and AF.Ln in v for v in tabs.values()):
                for name, s in tabs.items():
                    if not (AF.Exp in s and AF.Ln in s):
                        s.discard(AF.Exp)
                        s.discard(AF.Ln)
            return tabs

        hw_specs.get_activation_tables = doctored
        _bacc.get_activation_tables = doctored
        try:
            yield
        finally:
            hw_specs.get_activation_tables = real
            _bacc.get_activation_tables = real

    return ctx()


def _build(meta, enable_asserts=False):
    import concourse.bass as bass
    import concourse.tile as tile
    from concourse import bacc, bass_isa, mybir
    from concourse.bass import _add_dep_helper

    nj0, nj1, ns0, ns1 = meta
    njtot = nj0 + nj1
    nj0a = (nj0 * 7 + 5) // 10
    nj0b = nj0 - nj0a
    nj1a = (nj1 * 7 + 5) // 10
    nj1b = nj1 - nj1a
    ni = [nj0a * P, nj1a * P, nj0b * P + ns0, nj1b * P + ns1]
    ncols = [(-(-n // 16)) for n in ni]
    # out slots per call (sampled slot appended to calls 2 and 3)
    slots = [nj0a, nj1a, nj0b + 1, nj1b + 1]
    queues = [1, 3, 2, 2]
    AUXW = njtot + GMAX * njtot + GMAX
    AUXHW = njtot + ES

    f32 = mybir.dt.float32
    bf16 = mybir.dt.bfloat16
    i16 = mybir.dt.int16
    AF = mybir.ActivationFunctionType
    OP = mybir.AluOpType
    AX = mybir.AxisListType

    nc = bacc.Bacc(
        "TRN2",
        target_bir_lowering=False,
        debug=False,
        enable_asserts=enable_asserts,
        num_devices=NCORES,
        num_swdge_queues=4,
    )

    lT = nc.dram_tensor("lT", [C, RB], bf16, kind="ExternalInput").ap()
    lH = nc.dram_tensor("lH", [P, HW_], bf16, kind="ExternalInput").ap()
    idx16 = nc.dram_tensor("idx16", [P, sum(ncols)], i16,
                           kind="ExternalInput").ap()
    aux = nc.dram_tensor("aux", [P, AUXW], f32, kind="ExternalInput").ap()
    auxh = nc.dram_tensor("auxh", [P, AUXHW], bf16, kind="ExternalInput").ap()
    out = nc.dram_tensor("out", [1, 1], f32, kind="ExternalOutput").ap()

    with tile.TileContext(nc) as tc:
        with tc.tile_pool(name="sb", bufs=1) as sb:
            # --- tiles ---
            dummy_idx = sb.tile([P, 1], i16)
            gdummy = sb.tile([P, ES], bf16)
            idx16_t = sb.tile([P, sum(ncols)], i16)
            aux_t = sb.tile([P, AUXW], f32)
            auxh_t = sb.tile([P, AUXHW], bf16)
            gt = [sb.tile([P, s * ES], bf16, name=f"gt{k}")
                  for k, s in enumerate(slots)]
            ht = sb.tile([P, HW_], bf16)
            msk = sb.tile([P, njtot * ES], bf16)
            val = sb.tile([P, njtot], f32)

            # --- early memsets (vector) + small input DMAs (scalar ring) ---
            nc.vector.memset(dummy_idx[:, :], 0)
            nc.vector.memset(gt[2][:, nj0b * ES:], -50.0)
            nc.vector.memset(gt[3][:, nj1b * ES:], -50.0)
            nc.sync.dma_start(out=idx16_t[:, :], in_=idx16[:, :])
            nc.scalar.dma_start(out=auxh_t[:, :], in_=auxh[:, :])
            nc.scalar.dma_start(out=aux_t[:, :], in_=aux[:, :])

            # prime the single Exp+Ln act table load early (no data deps)
            prime = sb.tile([1, 1], f32)
            nc.vector.memset(prime[:, :], 0.0)
            nc.scalar.activation(prime[:, :], prime[:, :], AF.Exp, scale=0.0)

            o = 0
            wcorr_t = aux_t[:, o:o + njtot]; o += njtot
            wg_t = aux_t[:, o:o + GMAX * njtot]; o += GMAX * njtot
            rcnt_t = aux_t[:, o:o + GMAX]; o += GMAX
            offt_t = auxh_t[:, 0:njtot]
            iota_t = auxh_t[:, njtot:njtot + ES]

            # --- dummy gather first: pays the mlp library IRAM load while
            # the idx/aux DMAs are still in flight ---
            gdum = nc.gpsimd.dma_gather(
                out_ap=gdummy[:, :].rearrange("p (j e) -> p j e", e=ES),
                in_ap=lT[0:16, :], idxs_ap=dummy_idx[:, :],
                num_idxs=16, num_idxs_reg=16, elem_size=ES,
                single_packet=False)

            oc = 0
            los = [0, LO1, 0, LO1]
            gcalls = []
            for k in range(4):
                g = nc.gpsimd.dma_gather(
                    out_ap=gt[k][:, :].rearrange("p (j e) -> p j e", e=ES),
                    in_ap=lT[los[k]:los[k] + WIN, :],
                    idxs_ap=idx16_t[:, oc:oc + ncols[k]],
                    num_idxs=ni[k], num_idxs_reg=ni[k], elem_size=ES,
                    single_packet=False, queue_num=queues[k])
                gcalls.append(g)
                oc += ncols[k]

            # --- head DMA on sync ring, gated behind the library load so the
            # ucode image isn't bandwidth-starved ---
            d_h0 = nc.sync.dma_start(out=ht[:, :], in_=lH[:, :])

            # --- vector: eq masks (after the dummy so the DVE SBUF port
            # doesn't contend with the gpsimd library IRAM load), then
            # per-call extract: mult, bf16 fold 256->128, reduce ---
            eq = nc.vector.tensor_tensor(
                out=msk[:, :].rearrange("p (j e) -> p j e", e=ES),
                in0=iota_t.unsqueeze(1).to_broadcast([P, njtot, ES]),
                in1=offt_t.unsqueeze(2).to_broadcast([P, njtot, ES]),
                op=OP.is_equal)
            _add_dep_helper(eq.ins, gdum.ins, sync=False,
                            reason="eq masks after gpsimd lib load")
            fold = sb.tile([P, njtot * (ES // 2)], bf16)
            jos = [0, nj0, nj0a, nj0 + nj1a]
            cns = [nj0a, nj1a, nj0b, nj1b]
            for k in range(4):
                jo, cn = jos[k], cns[k]
                nc.vector.tensor_tensor(
                    msk[:, jo * ES:(jo + cn) * ES],
                    msk[:, jo * ES:(jo + cn) * ES],
                    gt[k][:, :cn * ES], op=OP.mult)
                mv = msk[:, jo * ES:(jo + cn) * ES].rearrange(
                    "p (j t e) -> p j t e", t=2, e=ES // 2)
                fv = fold[:, jo * (ES // 2):(jo + cn) * (ES // 2)].rearrange(
                    "p (j e) -> p j e", e=ES // 2).unsqueeze(2)
                nc.vector.tensor_tensor(
                    fv, mv[:, :, 0:1, :], mv[:, :, 1:2, :], op=OP.add)
                nc.vector.tensor_reduce(
                    val[:, jo:jo + cn],
                    fold[:, jo * (ES // 2):(jo + cn) * (ES // 2)].rearrange(
                        "p (j e) -> p j e", e=ES // 2),
                    AX.X, OP.add)

            # --- scalar: softplus everywhere (single Exp+Ln table set) ---
            hacc = sb.tile([P, 1], f32)
            e_h = nc.scalar.activation(ht[:, :], ht[:, :], AF.Exp)
            nc.scalar.activation(ht[:, :], ht[:, :], AF.Ln, bias=1.0,
                                 accum_out=hacc[:, :])
            sacc = [sb.tile([P, 1], f32, name=f"sacc{w}") for w in (0, 1)]
            for w, k, cn in ((0, 2, nj0b), (1, 3, nj1b)):
                e_s = nc.scalar.activation(gt[k][:, cn * ES:],
                                           gt[k][:, cn * ES:], AF.Exp)
                _add_dep_helper(e_s.ins, e_h.ins, sync=False,
                                reason="head softplus first on scalar queue")
                nc.scalar.activation(gt[k][:, cn * ES:], gt[k][:, cn * ES:],
                                     AF.Ln, bias=1.0, accum_out=sacc[w][:, :])

            # --- term1: per-row avg over candidates ---
            scr2 = sb.tile([P, GMAX * njtot], f32)
            for g in range(GMAX):
                nc.vector.tensor_tensor(
                    scr2[:, g * njtot:(g + 1) * njtot],
                    wg_t[:, g * njtot:(g + 1) * njtot], val[:, :], op=OP.mult)
            csum = sb.tile([P, GMAX], f32)
            nc.vector.tensor_reduce(
                csum[:, :],
                scr2[:, :].rearrange("p (g j) -> p g j", g=GMAX),
                AX.X, OP.add)
            avg = sb.tile([P, GMAX], f32)
            nc.vector.tensor_tensor(avg[:, :], csum[:, :], rcnt_t, op=OP.mult)

            # --- late activations ---
            ce = sb.tile([P, njtot], f32)
            nc.scalar.activation(ce[:, :], val[:, :], AF.Exp)
            ae = sb.tile([P, GMAX], f32)
            nc.scalar.activation(ae[:, :], avg[:, :], AF.Exp, scale=-1.0)
            spl = sb.tile([P, njtot], f32)
            nc.scalar.activation(spl[:, :], ce[:, :], AF.Ln, bias=1.0)
            t1c = sb.tile([P, 1], f32)
            t1 = sb.tile([P, GMAX], f32)
            nc.scalar.activation(t1[:, :], ae[:, :], AF.Ln, bias=1.0,
                                 accum_out=t1c[:, :])

            # --- combine ---
            corr = sb.tile([P, 1], f32)
            scr3 = sb.tile([P, njtot], f32)
            nc.vector.tensor_tensor(scr3[:, :], wcorr_t, spl[:, :], op=OP.mult)
            nc.vector.tensor_reduce(corr[:, :], scr3[:, :], AX.X, OP.add)

            total = sb.tile([P, 1], f32)
            nc.vector.tensor_tensor(total[:, :], t1c[:, :], corr[:, :],
                                    op=OP.add)
            stot = sb.tile([P, 1], f32)
            nc.vector.tensor_tensor(stot[:, :], sacc[0][:, :], sacc[1][:, :],
                                    op=OP.add)
            sacc2 = sb.tile([P, 1], f32)
            nc.vector.tensor_scalar_mul(sacc2[:, :], stot[:, :], TSCALE)
            nc.vector.tensor_tensor(total[:, :], total[:, :], sacc2[:, :],
                                    op=OP.add)
            nc.vector.tensor_tensor(total[:, :], total[:, :], hacc[:, :],
                                    op=OP.add)

            gtot = sb.tile([P, 1], f32)
            nc.gpsimd.partition_all_reduce(gtot[:, :], total[:, :],
                                           channels=P,
                                           reduce_op=bass_isa.ReduceOp.add)
            res = sb.tile([1, 1], f32)
            nc.vector.tensor_scalar_mul(res[:, :], gtot[0:1, :], 1.0 / B)
            nc.sync.dma_start(out=out[:, :], in_=res[:, :])

    with _act_table_patch():
        nc.compile()
    return nc


def get_graph(meta, enable_asserts=False):
    key = (meta, enable_asserts)
    if key not in _CACHE:
        _CACHE[key] = _build(meta, enable_asserts=enable_asserts)
    return _CACHE[key]


def run(logits, candidates, sampled_indices, trace=False, **kw):
    """Returns (scalar float32 loss, BassKernelResults)."""
    from concourse.bass_utils import run_bass_kernel_spmd

    in_maps, meta = prep_inputs(logits, candidates, sampled_indices)
    nc = get_graph(meta)
    res = run_bass_kernel_spmd(nc, in_maps, core_ids=list(range(NCORES)),
                               trace=trace, **kw)
    partials = [r["out"].reshape(()) for r in res.results]
    loss = np.float32(np.sum(np.stack(partials), dtype=np.float64))
    return loss, res


def kernel(logits, candidates, sampled_indices):
    loss, _ = run(logits, candidates, sampled_indices, trace=False)
    return loss


# revision 26
# speedup vs baseline: 1.0482x; 1.0482x over previous
"""AdaptiveCLPL loss on 8 TRN2 NeuronCores (Bass/Tile) — v6.

loss = mean_b [ psi(avg_cand) + sum_head psi(-l)*(1-mask) + ts*sum_samp psi(-l)*(1-is_cand) ]
with psi(u) = softplus(-u) = Ln(Exp(-u)+1) (no native softplus table).

Decomposition (only term1 is per-row nonlinear; everything else sums):
  total = sum_b softplus(-avg_b)
        + [sum_{head block} softplus(l)    - sum_k uniq*inhead*softplus(l_cand)]
        + ts*[sum_{sampled rows} softplus(l) - sum_k uniq*mult*softplus(l_cand)]

Per-core layout: transposed batch shard lT = logits[rows_perm].T in BF16
([C, RB] row-major); every lT row is a 512B chunk addressed by class (bf16
halves both the gather wire and the DVE extraction; the 2e-2 tolerance has
orders of magnitude of headroom). Candidate values come from dma_gather (one
descriptor per candidate). Key points:
  - overlapping int16 windows [0,32768) and [C-32768, C): candidates in the
    overlap go to either window, so every partition holds EXACTLY nj0+nj1
    candidate slots -> zero descriptor padding (2560 descriptors, the floor).
  - sampled rows ride the window gather calls as extra trailing indices,
    replacing the slow gpsimd indirect DMA.
  - 3 gather calls (w0 | w1a | w1b) across 2 SWDGE queues: each call's
    end-doorbell releases its wire while the next call's descriptors
    generate, so extraction pipelines with generation.
  - a 16-idx dummy gather issued first pays the gpsimd 'mlp' library IRAM
    load while the idx/aux DMAs are in flight; the 2MB head DMA is gated
    behind it so the library image isn't bandwidth-starved. The head input
    is reshaped host-side to 128 partitions so its DMA spreads across all
    16 SDMA engines (a [125, *] shape lands on only 5).
  - act tables are doctored at compile time so Exp and Ln resolve to the one
    table set that contains both -> one ACT_TABLE_LOAD, primed early by a
    dummy activation.
  - rows are packed 2 per partition; the shard column of row (p,g) is 2p+g.
"""

import numpy as np
import ml_dtypes

BF16 = ml_dtypes.bfloat16

B, C, K = 2048, 50000, 10
HEAD, S = 2000, 100
TSCALE = float(C - HEAD) / float(S)  # 480.0
NCORES = 8
RB = B // NCORES  # 256 rows per core
P = 128
ES = 256          # chunk = one lT row (512B in bf16)
WIN = 32768
LO1 = C - WIN     # 17232; window1 = [LO1, C)
GMAX = 2          # exactly 2 rows per partition
HW_ = HEAD * RB // P  # 4000 head elements per partition

_CACHE = {}


def _pack_rows(h0, h1, nj_target, rng):
    """Pair 2*P rows into P partitions s.t. per-partition hard-window counts
    stay <= nj_target. Returns part[r] in [0,P)."""
    nrows = len(h0)
    order = np.argsort(-h0, kind="stable")
    part = np.zeros(nrows, np.int64)
    for i in range(P):
        part[order[i]] = i
        part[order[nrows - 1 - i]] = i
    H0 = np.bincount(part, weights=h0, minlength=P)
    H1 = np.bincount(part, weights=h1, minlength=P)

    def viol(a0, a1):
        return max(a0 - nj_target, 0) + max(a1 - nj_target, 0)

    cur = sum(viol(H0[p], H1[p]) for p in range(P))
    it = 0
    while cur > 0 and it < 20000:
        it += 1
        a, b = rng.integers(0, nrows, 2)
        pa, pb = part[a], part[b]
        if pa == pb:
            continue
        old = viol(H0[pa], H1[pa]) + viol(H0[pb], H1[pb])
        H0[pa] += h0[b] - h0[a]; H1[pa] += h1[b] - h1[a]
        H0[pb] += h0[a] - h0[b]; H1[pb] += h1[a] - h1[b]
        new = viol(H0[pa], H1[pa]) + viol(H0[pb], H1[pb])
        if new <= old:
            part[a], part[b] = pb, pa
            cur += new - old
        else:
            H0[pa] -= h0[b] - h0[a]; H1[pa] -= h1[b] - h1[a]
            H0[pb] -= h0[a] - h0[b]; H1[pb] -= h1[a] - h1[b]
    return part, cur == 0


def prep_inputs(logits, candidates, sampled_indices):
    """Full inputs -> (in_maps, meta). Host work is sharding + index math only."""
    logits = np.asarray(logits)
    candidates = np.asarray(candidates)
    sampled_indices = np.asarray(sampled_indices)
    assert logits.shape == (B, C) and candidates.shape == (B, K)
    srow = (HEAD + sampled_indices.astype(np.int64)).astype(np.int64)  # [S]
    svals, scounts = np.unique(srow, return_counts=True)
    smult = dict(zip(svals.tolist(), scounts.tolist()))

    # sampled rows -> windows (balance the flexible ones)
    s_w = np.where(srow < LO1, 0, np.where(srow >= WIN, 1, -1))
    flex = np.where(s_w < 0)[0]
    n0 = int((s_w == 0).sum())
    n1 = int((s_w == 1).sum())
    for j in flex:
        if n0 <= n1:
            s_w[j] = 0; n0 += 1
        else:
            s_w[j] = 1; n1 += 1
    ns0, ns1 = n0, n1
    sidx_w = [srow[s_w == 0] - 0, srow[s_w == 1] - LO1]

    rng = np.random.default_rng(12345)
    cores = []
    nj_need = [1, 1]
    for i in range(NCORES):
        rows = slice(i * RB, (i + 1) * RB)
        cand = candidates[rows].astype(np.int64)          # [RB, K]
        valid = cand >= 0
        uniq = valid.copy()
        for k in range(1, K):
            dup = (cand[:, :k] == cand[:, k:k + 1]).any(axis=1)
            uniq[:, k] &= ~dup
        uniqf = uniq.astype(np.float32)
        cnt = np.maximum((uniq & valid).sum(axis=1), 1).astype(np.float32)
        inhead = (cand < HEAD).astype(np.float32)
        mult = np.vectorize(lambda c: smult.get(int(c), 0))(cand).astype(np.float32)
        wcorr_rk = -uniqf * (inhead + TSCALE * mult)      # [RB, K]

        h0 = (valid & (cand < LO1)).sum(axis=1)
        h1 = (valid & (cand >= WIN)).sum(axis=1)
        part, ok = _pack_rows(h0.astype(np.int64), h1.astype(np.int64), K, rng)
        grp = np.zeros(RB, np.int64)
        seen = {}
        for r in range(RB):
            p = int(part[r])
            grp[r] = seen.get(p, 0)
            seen[p] = grp[r] + 1
        assert max(seen.values()) <= GMAX

        # window assignment per candidate
        cw = np.full((RB, K), -1, np.int64)
        cw[valid & (cand < LO1)] = 0
        cw[valid & (cand >= WIN)] = 1
        H0 = np.bincount(part, weights=(cw == 0).sum(1), minlength=P).astype(np.int64)
        for r in range(RB):
            p = int(part[r])
            for k in range(K):
                if valid[r, k] and cw[r, k] < 0:
                    if H0[p] < K:
                        cw[r, k] = 0; H0[p] += 1
                    else:
                        cw[r, k] = 1
        W0c = np.bincount(part, weights=(cw == 0).sum(1), minlength=P).astype(np.int64)
        W1c = np.bincount(part, weights=(cw == 1).sum(1), minlength=P).astype(np.int64)
        nj_need[0] = max(nj_need[0], int(W0c.max()))
        nj_need[1] = max(nj_need[1], int(W1c.max()))
        cores.append((cand, valid, uniqf, cnt, wcorr_rk, part, grp, cw))

    nj0, nj1 = nj_need
    njtot = nj0 + nj1
    meta = (nj0, nj1, ns0, ns1)

    # gather calls: [w0a] [w1a] [w0b cand+samp0] [w1b cand+samp1]
    nj0a = (nj0 * 7 + 5) // 10
    nj0b = nj0 - nj0a
    nj1a = (nj1 * 7 + 5) // 10
    nj1b = nj1 - nj1a
    ni = [nj0a * P, nj1a * P, nj0b * P + ns0, nj1b * P + ns1]
    ncols = [(-(-n // 16)) for n in ni]
    AUXW = njtot + GMAX * njtot + GMAX          # f32: wcorr, wg, rcnt
    AUXHW = njtot + ES                          # bf16: offt, iota

    in_maps = []
    for i in range(NCORES):
        cand, valid, uniqf, cnt, wcorr_rk, part, grp, cw = cores[i]
        rows = slice(i * RB, (i + 1) * RB)
        col = (2 * part + grp).astype(np.int64)
        perm = np.zeros(RB, np.int64)
        perm[col] = np.arange(RB)
        lT = np.ascontiguousarray(
            logits[rows][perm].T.astype(np.float32).astype(BF16))
        lH = np.ascontiguousarray(lT[:HEAD].reshape(P, HW_))

        idxs = [np.zeros(nj0 * P, np.int16), np.zeros(nj1 * P, np.int16)]
        offt = np.full((P, njtot), -1.0, np.float32)
        wcorr = np.zeros((P, njtot), np.float32)
        wg = np.zeros((P, GMAX * njtot), np.float32)
        fill = np.zeros((P, 2), np.int64)
        base_j = [0, nj0]
        base_lo = [0, LO1]
        for r in range(RB):
            p, g = int(part[r]), int(grp[r])
            for k in range(K):
                if not valid[r, k]:
                    continue
                w = int(cw[r, k])
                j = int(fill[p, w]); fill[p, w] += 1
                idxs[w][j * P + p] = cand[r, k] - base_lo[w]
                jj = base_j[w] + j
                offt[p, jj] = float(col[r])
                wcorr[p, jj] = wcorr_rk[r, k]
                wg[p, g * njtot + jj] = uniqf[r, k]
        rcnt = np.zeros((P, GMAX), np.float32)
        rcnt[part, grp] = 1.0 / cnt
        iota = np.broadcast_to(np.arange(ES, dtype=np.float32), (P, ES)).copy()

        # per-call idx lists
        lists = [
            idxs[0][:nj0a * P],
            idxs[1][:nj1a * P],
            np.concatenate([idxs[0][nj0a * P:], sidx_w[0].astype(np.int16)]),
            np.concatenate([idxs[1][nj1a * P:], sidx_w[1].astype(np.int16)]),
        ]
        cols_out = []
        for li, n_c in zip(lists, ncols):
            flat = np.zeros(n_c * 16, np.int16)
            flat[:len(li)] = li
            wrapped = flat.reshape(n_c, 16).T
            cols_out.append(np.tile(wrapped, (8, 1)))
        idx16 = np.ascontiguousarray(np.concatenate(cols_out, axis=1))
        assert idx16.shape == (P, sum(ncols))

        auxcat = np.ascontiguousarray(np.concatenate(
            [wcorr, wg, rcnt], axis=1))
        assert auxcat.shape == (P, AUXW)
        auxh = np.ascontiguousarray(np.concatenate(
            [offt, iota], axis=1).astype(BF16))
        assert auxh.shape == (P, AUXHW)
        in_maps.append({"lT": lT, "lH": lH, "idx16": idx16,
                        "aux": auxcat, "auxh": auxh})
    return in_maps, meta


def _act_table_patch():
    """Context manager: make Exp and Ln resolve only to the one act-func set
    ('natural_log_exp_and_others') that holds both, so the compile-time table
    placement emits a single ACT_TABLE_LOAD instead of swapping per phase."""
    import contextlib as _ctl
    from concourse import hw_specs, mybir
    from concourse import bacc as _bacc

    @_ctl.contextmanager
    def ctx():
        real = hw_specs.get_activation_tables
        AF = mybir.ActivationFunctionType

        def doctored(arch):
            tabs = {k: set(v) for k, v in real(arch).items()}
            if any(AF.Exp in v and AF.Ln in v for v in tabs.values()):
                for name, s in tabs.items():
                    if not (AF.Exp in s and AF.Ln in s):
                        s.discard(AF.Exp)
                        s.discard(AF.Ln)
            return tabs

        hw_specs.get_activation_tables = doctored
        _bacc.get_activation_tables = doctored
        try:
            yield
        finally:
            hw_specs.get_activation_tables = real
            _bacc.get_activation_tables = real

    return ctx()


def _build(meta, enable_asserts=False):
    import concourse.bass as bass
    import concourse.tile as tile
    from concourse import bacc, bass_isa, mybir
    from concourse.bass import _add_dep_helper

    nj0, nj1, ns0, ns1 = meta
    njtot = nj0 + nj1
    nj0a = (nj0 * 7 + 5) // 10
    nj0b = nj0 - nj0a
    nj1a = (nj1 * 7 + 5) // 10
    nj1b = nj1 - nj1a
    ni = [nj0a * P, nj1a * P, nj0b * P + ns0, nj1b * P + ns1]
    ncols = [(-(-n // 16)) for n in ni]
    # out slots per call (sampled slot appended to calls 2 and 3)
    slots = [nj0a, nj1a, nj0b + 1, nj1b + 1]
    queues = [1, 3, 2, 2]
    AUXW = njtot + GMAX * njtot + GMAX
    AUXHW = njtot + ES

    f32 = mybir.dt.float32
    bf16 = mybir.dt.bfloat16
    i16 = mybir.dt.int16
    AF = mybir.ActivationFunctionType
    OP = mybir.AluOpType
    AX = mybir.AxisListType

    nc = bacc.Bacc(
        "TRN2",
        target_bir_lowering=False,
        debug=False,
        enable_asserts=enable_asserts,
        num_devices=NCORES,
        num_swdge_queues=4,
    )

    lT = nc.dram_tensor("lT", [C, RB], bf16, kind="ExternalInput").ap()
    lH = nc.dram_tensor("lH", [P, HW_], bf16, kind="ExternalInput").ap()
    idx16 = nc.dram_tensor("idx16", [P, sum(ncols)], i16,
                           kind="ExternalInput").ap()
    aux = nc.dram_tensor("aux", [P, AUXW], f32, kind="ExternalInput").ap()
    auxh = nc.dram_tensor("auxh", [P, AUXHW], bf16, kind="ExternalInput").ap()
    out = nc.dram_tensor("out", [1, 1], f32, kind="ExternalOutput").ap()

    with tile.TileContext(nc) as tc:
        with tc.tile_pool(name="sb", bufs=1) as sb:
            # --- tiles ---
            dummy_idx = sb.tile([P, 1], i16)
            gdummy = sb.tile([P, ES], bf16)
            idx16_t = sb.tile([P, sum(ncols)], i16)
            aux_t = sb.tile([P, AUXW], f32)
            auxh_t = sb.tile([P, AUXHW], bf16)
            gt = [sb.tile([P, s * ES], bf16, name=f"gt{k}")
                  for k, s in enumerate(slots)]
            ht = sb.tile([P, HW_], bf16)
            msk = sb.tile([P, njtot * ES], bf16)
            val = sb.tile([P, njtot], f32)

            # --- early memsets (vector) + small input DMAs (scalar ring) ---
            nc.vector.memset(dummy_idx[:, :], 0)
            nc.vector.memset(gt[2][:, nj0b * ES:], -50.0)
            nc.vector.memset(gt[3][:, nj1b * ES:], -50.0)
            nc.sync.dma_start(out=idx16_t[:, :], in_=idx16[:, :])
            nc.scalar.dma_start(out=auxh_t[:, :], in_=auxh[:, :])
            nc.scalar.dma_start(out=aux_t[:, :], in_=aux[:, :])

            # prime the single Exp+Ln act table load early (no data deps)
            prime = sb.tile([1, 1], f32)
            nc.vector.memset(prime[:, :], 0.0)
            nc.scalar.activation(prime[:, :], prime[:, :], AF.Exp, scale=0.0)

            o = 0
            wcorr_t = aux_t[:, o:o + njtot]; o += njtot
            wg_t = aux_t[:, o:o + GMAX * njtot]; o += GMAX * njtot
            rcnt_t = aux_t[:, o:o + GMAX]; o += GMAX
            offt_t = auxh_t[:, 0:njtot]
            iota_t = auxh_t[:, njtot:njtot + ES]

            # --- dummy gather first: pays the mlp library IRAM load while
            # the idx/aux DMAs are still in flight ---
            gdum = nc.gpsimd.dma_gather(
                out_ap=gdummy[:, :].rearrange("p (j e) -> p j e", e=ES),
                in_ap=lT[0:16, :], idxs_ap=dummy_idx[:, :],
                num_idxs=16, num_idxs_reg=16, elem_size=ES,
                single_packet=False)

            oc = 0
            los = [0, LO1, 0, LO1]
            gcalls = []
            for k in range(4):
                g = nc.gpsimd.dma_gather(
                    out_ap=gt[k][:, :].rearrange("p (j e) -> p j e", e=ES),
                    in_ap=lT[los[k]:los[k] + WIN, :],
                    idxs_ap=idx16_t[:, oc:oc + ncols[k]],
                    num_idxs=ni[k], num_idxs_reg=ni[k], elem_size=ES,
                    single_packet=False, queue_num=queues[k])
                gcalls.append(g)
                oc += ncols[k]

            # --- head DMA on sync ring, gated behind the library load so the
            # ucode image isn't bandwidth-starved ---
            d_h0 = nc.sync.dma_start(out=ht[:, :], in_=lH[:, :])
            _add_dep_helper(d_h0.ins, gdum.ins, sync=True,
                            reason="head DMA after gpsimd lib load")

            # --- vector: eq masks (after the dummy so the DVE SBUF port
            # doesn't contend with the gpsimd library IRAM load), then
            # per-call extract: mult, bf16 fold 256->128, reduce ---
            eq = nc.vector.tensor_tensor(
                out=msk[:, :].rearrange("p (j e) -> p j e", e=ES),
                in0=iota_t.unsqueeze(1).to_broadcast([P, njtot, ES]),
                in1=offt_t.unsqueeze(2).to_broadcast([P, njtot, ES]),
                op=OP.is_equal)
            _add_dep_helper(eq.ins, gdum.ins, sync=False,
                            reason="eq masks after gpsimd lib load")
            fold = sb.tile([P, njtot * (ES // 2)], bf16)
            jos = [0, nj0, nj0a, nj0 + nj1a]
            cns = [nj0a, nj1a, nj0b, nj1b]
            for k in range(4):
                jo, cn = jos[k], cns[k]
                nc.vector.tensor_tensor(
                    msk[:, jo * ES:(jo + cn) * ES],
                    msk[:, jo * ES:(jo + cn) * ES],
                    gt[k][:, :cn * ES], op=OP.mult)
                mv = msk[:, jo * ES:(jo + cn) * ES].rearrange(
                    "p (j t e) -> p j t e", t=2, e=ES // 2)
                fv = fold[:, jo * (ES // 2):(jo + cn) * (ES // 2)].rearrange(
                    "p (j e) -> p j e", e=ES // 2).unsqueeze(2)
                nc.vector.tensor_tensor(
                    fv, mv[:, :, 0:1, :], mv[:, :, 1:2, :], op=OP.add)
                nc.vector.tensor_reduce(
                    val[:, jo:jo + cn],
                    fold[:, jo * (ES // 2):(jo + cn) * (ES // 2)].rearrange(
                        "p (j e) -> p j e", e=ES // 2),
                    AX.X, OP.add)

            # --- scalar: softplus everywhere (single Exp+Ln table set) ---
            hacc = sb.tile([P, 1], f32)
            e_h = nc.scalar.activation(ht[:, :], ht[:, :], AF.Exp)
            nc.scalar.activation(ht[:, :], ht[:, :], AF.Ln, bias=1.0,
                                 accum_out=hacc[:, :])
            sacc = [sb.tile([P, 1], f32, name=f"sacc{w}") for w in (0, 1)]
            for w, k, cn in ((0, 2, nj0b), (1, 3, nj1b)):
                e_s = nc.scalar.activation(gt[k][:, cn * ES:],
                                           gt[k][:, cn * ES:], AF.Exp)
                _add_dep_helper(e_s.ins, e_h.ins, sync=False,
                                reason="head softplus first on scalar queue")
                nc.scalar.activation(gt[k][:, cn * ES:], gt[k][:, cn * ES:],
                                     AF.Ln, bias=1.0, accum_out=sacc[w][:, :])

            # --- term1 partials per call as val slices land ---
            scr2 = sb.tile([P, GMAX * njtot], f32)
            ce = sb.tile([P, njtot], f32)
            spl = sb.tile([P, njtot], f32)
            scr3 = sb.tile([P, njtot], f32)
            for k in range(4):
                jo, cn = jos[k], cns[k]
                for g in range(GMAX):
                    nc.vector.tensor_tensor(
                        scr2[:, g * njtot + jo:g * njtot + jo + cn],
                        wg_t[:, g * njtot + jo:g * njtot + jo + cn],
                        val[:, jo:jo + cn], op=OP.mult)
                nc.scalar.activation(ce[:, jo:jo + cn], val[:, jo:jo + cn],
                                     AF.Exp)
                nc.scalar.activation(spl[:, jo:jo + cn], ce[:, jo:jo + cn],
                                     AF.Ln, bias=1.0)
                nc.vector.tensor_tensor(scr3[:, jo:jo + cn],
                                        wcorr_t[:, jo:jo + cn],
                                        spl[:, jo:jo + cn], op=OP.mult)
            csum = sb.tile([P, GMAX], f32)
            nc.vector.tensor_reduce(
                csum[:, :],
                scr2[:, :].rearrange("p (g j) -> p g j", g=GMAX),
                AX.X, OP.add)
            avg = sb.tile([P, GMAX], f32)
            nc.vector.tensor_tensor(avg[:, :], csum[:, :], rcnt_t, op=OP.mult)

            # --- late activations ---
            ae = sb.tile([P, GMAX], f32)
            nc.scalar.activation(ae[:, :], avg[:, :], AF.Exp, scale=-1.0)
            t1c = sb.tile([P, 1], f32)
            t1 = sb.tile([P, GMAX], f32)
            nc.scalar.activation(t1[:, :], ae[:, :], AF.Ln, bias=1.0,
                                 accum_out=t1c[:, :])

            # --- combine ---
            corr = sb.tile([P, 1], f32)
            nc.vector.tensor_reduce(corr[:, :], scr3[:, :], AX.X, OP.add)

            total = sb.tile([P, 1], f32)
            nc.vector.tensor_tensor(total[:, :], t1c[:, :], corr[:, :],
                                    op=OP.add)
            stot = sb.tile([P, 1], f32)
            nc.vector.tensor_tensor(stot[:, :], sacc[0][:, :], sacc[1][:, :],
                                    op=OP.add)
            sacc2 = sb.tile([P, 1], f32)
            nc.vector.tensor_scalar_mul(sacc2[:, :], stot[:, :], TSCALE)
            nc.vector.tensor_tensor(total[:, :], total[:, :], sacc2[:, :],
                                    op=OP.add)
            nc.vector.tensor_tensor(total[:, :], total[:, :], hacc[:, :],
                                    op=OP.add)

            gtot = sb.tile([P, 1], f32)
            nc.gpsimd.partition_all_reduce(gtot[:, :], total[:, :],
                                           channels=P,
                                           reduce_op=bass_isa.ReduceOp.add)
            res = sb.tile([1, 1], f32)
            nc.vector.tensor_scalar_mul(res[:, :], gtot[0:1, :], 1.0 / B)
            nc.sync.dma_start(out=out[:, :], in_=res[:, :])

    with _act_table_patch():
        nc.compile()
    return nc


def get_graph(meta, enable_asserts=False):
    key = (meta, enable_asserts)
    if key not in _CACHE:
        _CACHE[key] = _build(meta, enable_asserts=enable_asserts)
    return _CACHE[key]


def run(logits, candidates, sampled_indices, trace=False, **kw):
    """Returns (scalar float32 loss, BassKernelResults)."""
    from concourse.bass_utils import run_bass_kernel_spmd

    in_maps, meta = prep_inputs(logits, candidates, sampled_indices)
    nc = get_graph(meta)
    res = run_bass_kernel_spmd(nc, in_maps, core_ids=list(range(NCORES)),
                               trace=trace, **kw)
    partials = [r["out"].reshape(()) for r in res.results]
    loss = np.float32(np.sum(np.stack(partials), dtype=np.float64))
    return loss, res


def kernel(logits, candidates, sampled_indices):
    loss, _ = run(logits, candidates, sampled_indices, trace=False)
    return loss
